# revision 5
# baseline (speedup 1.0000x reference)
"""nn_GTN_58205396795517: 2-layer TransformerConv GNN on 8 NeuronCores.

Bass/Tile kernel. Strategy:
  - Destination-shard nodes across the 8 cores (12544 nodes/core); each core
    owns all edges into its node range, so the segment softmax/scatter-add
    needs no cross-core reduction.
  - Node features are kept in a replicated bf16 table; layer-1 needs no
    collective, layer-2's h table is AllGathered once inside the NEFF.
  - Edges are grouped into (128-node window, 64-node half-window) tiles of
    128 edges; per tile ONE indirect DMA gathers the source rows (144 B
    each).
  - Attention logits come from an all-pairs PE matmul B = x_src @ G_w per
    half-window (G_w folds Wk Wq^T and the window's x_dst rows); exp runs on
    the scalar engine over all of B; a one-hot slot mask (DVE) keeps each
    edge's own logit, and one PE matmul per tile segment-reduces the
    attention-weighted *source features* (Wv is applied after aggregation,
    per window, using linearity).
  - All matmul operands are bf16 (PSUM accumulation stays fp32).

Falls back to a pure-JAX shard_map implementation if the Bass path fails.
"""
import numpy as np
import ml_dtypes

NC = 8
N = 100000
SH = 12544            # nodes per core (128-aligned)
NPAD = SH * NC        # 100352
D = 64
H = 4
WIN = SH // 128       # 98 windows per core
NG = WIN * 2          # 196 half-window groups per core
ROW = 72              # padded bf16 node-table row (64 feat | 1.0 | pad)
AW = 65

BF16 = ml_dtypes.bfloat16

_state = {}


# ----------------------------------------------------------------------------
# walrus workarounds (wait-split + Tile tail drain)
# ----------------------------------------------------------------------------
def _install_patches():
    import json as _json
    import os as _os
    from concourse import bass_utils, bass2jax, tile
    from concourse.vector_clock import ScopedClock

    if getattr(bass_utils, "_gtn_patch_installed", False):
        return
    bass_utils._gtn_patch_installed = True

    MAXW = 1

    def _split_waits(bir):
        changed = False
        for fn in bir.get("functions", []):
            for bb in fn.get("blocks", []):
                insts = bb.get("instructions")
                if not insts:
                    continue
                out = []
                for inst in insts:
                    si = inst.get("sync_info") or {}
                    waits = si.get("on_wait") or []
                    if len(waits) > MAXW:
                        changed = True
                        extra, keep = waits[:-MAXW], waits[-MAXW:]
                        for n, i0 in enumerate(range(0, len(extra), MAXW)):
                            out.append({
                                "name": f"{inst['name']}_wsplit{n}",
                                "opcode": "NoOp",
                                "engine": inst.get("engine"),
                                "ins": [], "outs": [],
                                "sync_info": {"on_wait": extra[i0:i0 + MAXW],
                                              "on_update": []},
                            })
                        si = dict(si); si["on_wait"] = keep
                        inst = dict(inst); inst["sync_info"] = si
                    out.append(inst)
                bb["instructions"] = out
        return changed

    orig = bass_utils.compile_bir_kernel

    def patched(bir_json, tmpdir, neff_name="file.neff"):
        bir = _json.loads(bir_json)
        if _split_waits(bir):
            bir_json = _json.dumps(bir).encode()
        return orig(bir_json, tmpdir, neff_name=neff_name)

    bass_utils.compile_bir_kernel = patched
    bass2jax.compile_bir_kernel = patched

    def _drain_and_barrier(self, tick_clock, wait_clock):
        nop_inst = self.nc.sync.nop(nofuse=True)
        wait_clock.add_sem_waits(
            nop_inst.ins, ScopedClock({None: tick_clock.global_clock}))
        self.nc.sync.drain()
        self.nc.all_engine_barrier()
        assert self.sems is not None
        popped = self.nc._tile_sem_poison_stack.pop()
        assert popped is self._sem_poison
        self.nc.clear_and_free_semaphores(list(self.sems.allocated().values()))
        self.nc.all_engine_barrier()

    tile.TileContext._drain_and_barrier = _drain_and_barrier


# ----------------------------------------------------------------------------
# host-side preprocessing
# ----------------------------------------------------------------------------
def _prep_edges(edge_index):
    src = np.asarray(edge_index[0], np.int64)
    dst = np.asarray(edge_index[1], np.int64)
    TW = 9                       # 4 cols per half + 1 shared overflow col
    cols = WIN * TW
    isrc = np.zeros((NC, 128, cols), np.int32)
    slot = np.full((NC, 128, cols), -1.0, np.float32)
    ok = True
    for c in range(NC):
        lo = c * SH
        m = (dst >= lo) & (dst < lo + SH)
        s_ = src[m]
        d_ = dst[m] - lo
        w_ = d_ >> 7
        for w in range(WIN):
            mw = w_ == w
            sw, dw = s_[mw], d_[mw]
            half = (dw >> 6) & 1
            parts = []
            for hh in (0, 1):
                sh_, dh_ = sw[half == hh], dw[half == hh]
                n = len(sh_)
                cap = min(n, 512)
                for t in range((cap + 127) // 128):
                    a, b = t * 128, min((t + 1) * 128, cap)
                    col = w * TW + hh * 4 + t
                    isrc[c, 0:b - a, col] = sh_[a:b]
                    slot[c, 0:b - a, col] = (dh_[a:b] & 127).astype(np.float32)
                parts.append((sh_[cap:], dh_[cap:]))
            so = np.concatenate([p[0] for p in parts])
            do = np.concatenate([p[1] for p in parts])
            if len(so) > 128:
                ok = False
                break
            col = w * TW + 8
            isrc[c, 0:len(so), col] = so
            slot[c, 0:len(so), col] = (do & 127).astype(np.float32)
        if not ok:
            break
    if not ok:
        raise RuntimeError("overflow-tile layout does not fit this graph")
    return isrc, slot.astype(BF16), 4


def _prep_weights(Wq, bq, Wk, bk, Wv, bv, Ws, bs):
    Wq = np.asarray(Wq, np.float32); Wk = np.asarray(Wk, np.float32)
    Wv = np.asarray(Wv, np.float32); Ws = np.asarray(Ws, np.float32)
    bq = np.asarray(bq, np.float32); bv = np.asarray(bv, np.float32)
    bs = np.asarray(bs, np.float32)
    # A[:, h*65:(h+1)*65]: lhsT for G_h = A_h^T-applied to xowT.
    #   A_h[k, c] (k<64) = Wtil_h[c, k],  A_h[64, c] = (Wk_h @ bq_h)[c]
    A = np.zeros((AW, H * AW), np.float32)
    for h in range(H):
        Wq_h = Wq[:, h * D:(h + 1) * D]
        Wk_h = Wk[:, h * D:(h + 1) * D]
        Wtil = Wk_h @ Wq_h.T            # [c, d]
        A[0:D, h * AW:h * AW + D] = Wtil.T
        A[D, h * AW:h * AW + D] = Wk_h @ bq_h_or_zero(bq, h)
    # WvS: [65, H*64]: per-head Wv/H padded with a zero row at index 64
    # (rhs for M_h = pswS_h^T-as-lhsT @ WvS_h; row 64 keeps den out of M).
    WvS = np.zeros((AW, H * D), np.float32)
    for h in range(H):
        WvS[0:D, h * D:(h + 1) * D] = Wv[:, h * D:(h + 1) * D] / H
    bv_mean = bv.reshape(H, D).mean(axis=0)
    ws_aug = np.concatenate([Ws, bs[None, :] + bv_mean[None, :] * 0.0], axis=0)
    # NOTE: bv handled only when zero (harness case); assert to be safe.
    assert np.abs(bv).max() == 0.0, "nonzero bv not supported by fast path"
    return A.astype(BF16), WvS.astype(BF16), ws_aug.astype(BF16)


def bq_h_or_zero(bq, h):
    return bq[h * D:(h + 1) * D]


# ----------------------------------------------------------------------------
# Bass program
# ----------------------------------------------------------------------------
def _build_nc(t_h, w_limit=WIN, layers=(1, 2), ablate=()):
    import concourse.bass as bass
    import concourse.mybir as mybir
    import concourse.tile as tile
    from concourse.masks import make_identity

    f32 = mybir.dt.float32
    bf16 = mybir.dt.bfloat16
    i32 = mybir.dt.int32
    COLS = WIN * 9
    WCOL = 9                     # gather columns per window

    nc = bass.Bass()
    xfull = nc.dram_tensor("xfull", [NPAD, ROW], bf16, kind="ExternalInput")
    xsh = nc.dram_tensor("xsh", [SH, ROW], bf16, kind="ExternalInput")
    isrc = nc.dram_tensor("isrc", [128, COLS], i32, kind="ExternalInput")
    slot = nc.dram_tensor("slot", [128, COLS], bf16, kind="ExternalInput")
    iota64 = nc.dram_tensor("iota64", [128, 128], bf16, kind="ExternalInput")
    hot4 = nc.dram_tensor("hot4", [128, H * H], bf16, kind="ExternalInput")
    wA = {}
    wV = {}
    wS = {}
    for L in (1, 2):
        wA[L] = nc.dram_tensor(f"wa{L}", [AW, H * AW], bf16, kind="ExternalInput")
        wV[L] = nc.dram_tensor(f"wv{L}", [AW, H * D], bf16, kind="ExternalInput")
        wS[L] = nc.dram_tensor(f"ws{L}", [AW, D], bf16, kind="ExternalInput")
    out_sh = nc.dram_tensor("out_sh", [SH, D], f32, kind="ExternalOutput")

    hown = nc.dram_tensor("hown", [SH, ROW], bf16)
    hbounce = nc.dram_tensor("hbounce", [SH, ROW], bf16)
    hfull = nc.dram_tensor("hfull", [NPAD, ROW], bf16, addr_space="Shared")

    groups = [list(range(NC))]

    with tile.TileContext(nc) as tc:
        with tc.tile_pool(name="const", bufs=1) as cp, \
             tc.tile_pool(name="idxp", bufs=1) as ip, \
             tc.tile_pool(name="gat", bufs=6) as gp, \
             tc.tile_pool(name="work", bufs=6) as wp, \
             tc.tile_pool(name="winp", bufs=3) as wnp, \
             tc.tile_pool(name="pB", bufs=2, space="PSUM") as pBp, \
             tc.tile_pool(name="pT", bufs=2, space="PSUM") as pTp, \
             tc.tile_pool(name="pG", bufs=1, space="PSUM") as pGp, \
             tc.tile_pool(name="pW", bufs=2, space="PSUM") as pWp, \
             tc.tile_pool(name="pO", bufs=1, space="PSUM") as pOp:

            ident = cp.tile([128, 128], bf16)
            make_identity(nc, ident[:])
            iota_t = cp.tile([128, 128], bf16)
            nc.sync.dma_start(out=iota_t[:], in_=iota64[:])
            hot4_t = cp.tile([128, H * H], bf16)
            nc.sync.dma_start(out=hot4_t[:], in_=hot4[:])
            wA_t = {}
            wV_t = {}
            wS_t = {}
            for L in (1, 2):
                wA_t[L] = cp.tile([AW, H * AW], bf16, tag=f"wa{L}", name=f"wa{L}_t")
                nc.sync.dma_start(out=wA_t[L][:], in_=wA[L][:])
                wV_t[L] = cp.tile([AW, H * D], bf16, tag=f"wv{L}", name=f"wv{L}_t")
                nc.sync.dma_start(out=wV_t[L][:], in_=wV[L][:])
                wS_t[L] = cp.tile([AW, D], bf16, tag=f"ws{L}", name=f"ws{L}_t")
                nc.sync.dma_start(out=wS_t[L][:], in_=wS[L][:])
            isrc_t = ip.tile([128, COLS], i32)
            nc.sync.dma_start(out=isrc_t[:], in_=isrc[:])
            slot_t = ip.tile([128, COLS], bf16)
            nc.sync.dma_start(out=slot_t[:], in_=slot[:])

            def layer(L, table, own, relu, last):
                for w in range(w_limit):
                    # ---- window setup ----
                    xow = wnp.tile([128, ROW], bf16, tag="xow")
                    nc.sync.dma_start(out=xow[:],
                                      in_=own[w * 128:(w + 1) * 128, :])
                    xs_win = gp.tile([128, WCOL * ROW], bf16, tag="xsw")
                    if "nogather" in ablate:
                        for kk in range(WCOL):
                            nc.sync.dma_start(
                                out=xs_win[:, kk * ROW:(kk + 1) * ROW],
                                in_=table[0:128, :])
                    else:
                        for kk in range(WCOL):
                            col0 = w * WCOL + kk
                            nc.gpsimd.indirect_dma_start(
                                out=xs_win[:, kk * ROW:(kk + 1) * ROW],
                                out_offset=None, in_=table[:],
                                in_offset=bass.IndirectOffsetOnAxis(
                                    ap=isrc_t[:, col0:col0 + 1], axis=0))
                    pxo = pTp.tile([AW, 128], bf16, tag="pt")
                    nc.tensor.transpose(out=pxo[:], in_=xow[:, 0:AW],
                                        identity=ident[:])
                    xowT = wnp.tile([AW, 128], bf16, tag="xowT")
                    nc.vector.tensor_copy(out=xowT[:], in_=pxo[:])
                    pg = pGp.tile([AW, H * 128], f32, tag="pg")
                    for h in range(H):
                        nc.tensor.matmul(
                            out=pg[:, h * 128:(h + 1) * 128],
                            lhsT=wA_t[L][:, h * AW:(h + 1) * AW],
                            rhs=xowT[:], start=True, stop=True)
                    # G_sb layout: [AW, half, head, 64]
                    G_sb = wnp.tile([AW, H * 128], bf16, tag="gsb")
                    nc.scalar.activation(
                        out=G_sb[:].rearrange("p (s h n) -> p s h n", s=2, n=64),
                        in_=pg[:].rearrange("p (h s n) -> p s h n", s=2, n=64),
                        func=mybir.ActivationFunctionType.Copy)

                    psw = {}
                    psw[0] = pWp.tile([AW, H * 64], f32, tag="pw", name="psw0")
                    psw[1] = pWp.tile([AW, H * 64], f32, tag="pw", name="psw1")

                    def tile_body(ks, half, start, stop, xsT_in=None):
                        col = w * WCOL + ks
                        xs = xs_win[:, ks * ROW:ks * ROW + ROW]
                        smat = wp.tile([128, 64], bf16, tag="smat")
                        nc.vector.tensor_tensor(
                            out=smat[:],
                            in0=slot_t[:, col:col + 1].to_broadcast([128, 64]),
                            in1=iota_t[:, half * 64:(half + 1) * 64],
                            op=mybir.AluOpType.is_equal)
                        if xsT_in is None:
                            pst = pTp.tile([AW, 128], bf16, tag="pt")
                            nc.tensor.transpose(out=pst[:], in_=xs[0:128, 0:AW],
                                                identity=ident[:])
                            xsT = wp.tile([AW, 128], bf16, tag="xsT")
                            nc.scalar.activation(
                                out=xsT[:], in_=pst[:],
                                func=mybir.ActivationFunctionType.Copy)
                        else:
                            xsT = xsT_in
                        pB = pBp.tile([128, H * 64], f32, tag="pb")
                        nc.tensor.matmul(
                            out=pB[:], lhsT=xsT[:],
                            rhs=G_sb[:, half * 256:(half + 1) * 256],
                            start=True, stop=True)
                        EB = wp.tile([128, H * 64], bf16, tag="eb")
                        nc.scalar.activation(
                            out=EB[:], in_=pB[:],
                            func=mybir.ActivationFunctionType.Exp,
                            scale=0.125)
                        P = wp.tile([128, H * 64], bf16, tag="pp")
                        nc.vector.tensor_tensor(
                            out=P[:].rearrange("p (h n) -> p h n", n=64),
                            in0=EB[:].rearrange("p (h n) -> p h n", n=64),
                            in1=smat[:].rearrange("p (o n) -> p o n", o=1)
                                .to_broadcast([128, H, 64]),
                            op=mybir.AluOpType.mult)
                        nc.tensor.matmul(
                            out=psw[half][:], lhsT=xs[0:128, 0:AW],
                            rhs=P[:], start=start, stop=stop)
                        return xsT

                    for half in (0, 1):
                        for t in range(4):
                            tile_body(half * 4 + t, half, start=(t == 0),
                                      stop=False)
                    # shared overflow tile: one gather, both halves' masks
                    xsT_of = tile_body(8, 0, start=False, stop=True)
                    tile_body(8, 1, start=False, stop=True, xsT_in=xsT_of)

                    # ---- window finalize ----
                    # pswT[c, (h, n)] per half; den at row D.
                    pswS = {}
                    for half in (0, 1):
                        pswS[half] = wnp.tile([AW, H * 64], bf16,
                                              tag=f"psws{half}",
                                              name=f"pswS{half}")
                        nc.vector.tensor_copy(out=pswS[half][:],
                                              in_=psw[half][:])
                    # finalize PSUM tile: M [0:256] | den [256:260] | skip [260:324]
                    pF = pOp.tile([128, 324], f32, tag="pf")
                    for half in (0, 1):
                        for h in range(H):
                            nc.tensor.matmul(
                                out=pF[half * 64:(half + 1) * 64,
                                       h * 64:(h + 1) * 64],
                                lhsT=pswS[half][:, h * 64:(h + 1) * 64],
                                rhs=wV_t[L][0:AW, h * D:(h + 1) * D],
                                start=True, stop=True)
                    for half in (0, 1):
                        for h in range(H):
                            nc.tensor.matmul(
                                out=pF[half * 64:(half + 1) * 64, 256:260],
                                lhsT=pswS[half][D:D + 1, h * 64:(h + 1) * 64],
                                rhs=hot4_t[D:D + 1, h * H:(h + 1) * H],
                                start=(h == 0), stop=(h == H - 1))
                    nc.tensor.matmul(out=pF[:, 260:324], lhsT=xowT[:],
                                     rhs=wS_t[L][:], start=True, stop=True)
                    denb = wnp.tile([128, H], f32, tag="denb")
                    nc.scalar.activation(
                        out=denb[:], in_=pF[:, 256:260],
                        func=mybir.ActivationFunctionType.Copy,
                        bias=1e-12)
                    rd = wnp.tile([128, H], f32, tag="rd")
                    nc.vector.reciprocal(out=rd[:], in_=denb[:])
                    mt = wnp.tile([128, H * D], bf16, tag="mt")
                    nc.vector.tensor_tensor(
                        out=mt[:].rearrange("p (h c) -> p h c", c=D),
                        in0=pF[:, 0:256].rearrange("p (h c) -> p h c", c=D),
                        in1=rd[:].rearrange("p (h o) -> p h o", o=1)
                            .to_broadcast([128, H, D]),
                        op=mybir.AluOpType.mult)
                    hpart = wnp.tile([128, D], f32, tag="hpart")
                    nc.vector.tensor_reduce(
                        out=hpart[:],
                        in_=mt[:].rearrange("p (h c) -> p c h", c=D),
                        axis=mybir.AxisListType.X, op=mybir.AluOpType.add)
                    pout = wnp.tile([128, D], f32, tag="pog")
                    nc.vector.tensor_add(out=pout[:], in0=hpart[:],
                                         in1=pF[:, 260:324])
                    if last and "dumph" in ablate:
                        nc.vector.tensor_copy(out=pout[:], in_=xow[:, 0:D])
                    if last and "dumpcol" in ablate:
                        nc.vector.tensor_copy(out=pout[:], in_=xow[:, 8:ROW])
                    if last and "dumpf" in ablate:
                        xfw = wnp.tile([128, ROW], bf16, tag="xfw")
                        nc.sync.dma_start(out=xfw[:],
                                          in_=table[w * 128:(w + 1) * 128, :])
                        nc.vector.tensor_copy(out=pout[:], in_=xfw[:, 8:ROW])
                    if last:
                        nc.sync.dma_start(
                            out=out_sh[w * 128:(w + 1) * 128, :], in_=pout[:])
                    else:
                        hsum = wnp.tile([128, ROW], bf16, tag="hsum")
                        nc.scalar.activation(
                            out=hsum[:, 0:D], in_=pout[:],
                            func=(mybir.ActivationFunctionType.Relu if relu
                                  else mybir.ActivationFunctionType.Copy))
                        # pad cols: [1, 0 x7] via iota compare (keeps Pool free)
                        nc.vector.tensor_tensor(
                            out=hsum[:, D:ROW], in0=iota_t[:, 0:ROW - D],
                            in1=iota_t[:, 0:1].to_broadcast([128, ROW - D]),
                            op=mybir.AluOpType.is_equal)
                        nc.sync.dma_start(
                            out=hown[w * 128:(w + 1) * 128, :], in_=hsum[:])
                        nc.sync.dma_start(
                            out=hbounce[w * 128:(w + 1) * 128, :], in_=hsum[:])

            if layers == (1,):
                layer(1, xfull, xsh, relu=True, last=True)
            elif 1 in layers:
                layer(1, xfull, xsh, relu=True, last=False)
            if 2 in layers:
                nc.gpsimd.collective_compute(
                    "AllGather", mybir.AluOpType.bypass, replica_groups=groups,
                    ins=[hbounce[:]], outs=[hfull[:]])
            if 2 in layers:
                if "l2x" in ablate:
                    layer(2, xfull, xsh, relu=False, last=True)
                else:
                    layer(2, hfull, hown, relu=False, last=True)
            elif not layers:
                osb0 = cp.tile([128, D], f32)
                nc.gpsimd.memset(osb0[:], 0.0)
                nc.sync.dma_start(out=out_sh[0:128, :], in_=osb0[:])

    return nc


# ----------------------------------------------------------------------------
# SPMD runner (cached jitted executable)
# ----------------------------------------------------------------------------
class _Runner:
    def __init__(self, nc, donate=True):
        import jax
        from jax.sharding import Mesh, PartitionSpec, NamedSharding
        try:
            from jax.experimental.shard_map import shard_map
        except ImportError:
            from jax import shard_map
        from concourse.bass2jax import (_bass_exec_p, install_neuronx_cc_hook,
                                        partition_id_tensor)
        import concourse.mybir as mybir

        install_neuronx_cc_hook()
        self.jax = jax
        self.nc = nc
        partition_name = (nc.partition_id_tensor.name
                          if nc.partition_id_tensor else None)
        in_names, out_names, out_avals = [], [], []
        zero_outs = []
        for alloc in nc.m.functions[0].allocations:
            if not isinstance(alloc, mybir.MemoryLocationSet):
                continue
            name = alloc.memorylocations[0].name
            if alloc.kind == "ExternalInput":
                if name != partition_name:
                    in_names.append(name)
            elif alloc.kind == "ExternalOutput":
                shape = tuple(alloc.tensor_shape)
                dtype = mybir.dt.np(alloc.dtype)
                out_names.append(name)
                out_avals.append(jax.core.ShapedArray(shape, dtype))
                zero_outs.append(np.zeros(shape, dtype))
        self.in_names, self.out_names = in_names, out_names
        self.out_avals, self.zero_outs = out_avals, zero_outs
        n_params, n_outs = len(in_names), len(out_names)
        all_in = in_names + out_names + ([partition_name] if partition_name else [])

        def _body(*args):
            operands = list(args)
            if partition_name is not None:
                operands.append(partition_id_tensor())
            return tuple(_bass_exec_p.bind(
                *operands, out_avals=tuple(out_avals), in_names=tuple(all_in),
                out_names=tuple(out_names), lowering_input_output_aliases=(),
                sim_require_finite=False, sim_require_nnan=False, nc=nc))

        devices = jax.devices()[:NC]
        self.mesh = Mesh(np.asarray(devices), ("core",))
        self.sh = NamedSharding(self.mesh, PartitionSpec("core"))
        kwargs = dict(keep_unused=True)
        if donate:
            kwargs["donate_argnums"] = tuple(range(n_params, n_params + n_outs))
        self.donate = donate
        self.fn = jax.jit(shard_map(
            _body, mesh=self.mesh,
            in_specs=(PartitionSpec("core"),) * (n_params + n_outs),
            out_specs=(PartitionSpec("core"),) * n_outs, check_rep=False),
            **kwargs)
        self.n_params = n_params
        self._compiled = None

    def compiled(self, dev_inputs, zeros):
        """AOT-compile with the bass effect suppressed (C++ fast dispatch)."""
        if self._compiled is None:
            from concourse.bass2jax import fast_dispatch_compile
            args = [self.jax.ShapeDtypeStruct(a.shape, a.dtype)
                    for a in (*dev_inputs, *zeros)]
            self._compiled = fast_dispatch_compile(
                lambda: self.fn.lower(*args).compile())
        return self._compiled

    def device_inputs(self, in_maps):
        concat = [
            np.concatenate([np.asarray(m[name]) for m in in_maps], axis=0)
            for name in self.in_names
        ]
        return [self.jax.device_put(a, self.sh) for a in concat]

    def zeros(self):
        return [self.jax.device_put(
            np.zeros((NC * z.shape[0], *z.shape[1:]), z.dtype), self.sh)
            for z in self.zero_outs]

    def run(self, dev_inputs):
        outs = self.fn(*dev_inputs, *self.zeros())
        return [np.asarray(o) for o in outs]


# ----------------------------------------------------------------------------
# public entry
# ----------------------------------------------------------------------------
def _warmup(t_h, in_maps):
    """The first executable loaded in a process stays ~40ms/call slower on
    this axon setup; burn that slot with a windowless throwaway kernel."""
    if "warm" in _state:
        return
    nc = _build_nc(t_h, w_limit=0, layers=())
    r = _Runner(nc, donate=False)
    dev = r.device_inputs(in_maps)
    import jax
    jax.block_until_ready(r.fn(*dev, *r.zeros()))
    _state["warm"] = True


def _prepare(x, edge_index, weights, build_kw=None):
    """Returns (runner, in_maps)."""
    isrc, slot, t_h = _prep_edges(edge_index)
    build_kw = build_kw or {}
    key = ("bass2", t_h, tuple(sorted(build_kw.items())))
    if key not in _state:
        _install_patches()
        nc = _build_nc(t_h, **build_kw)
        _state[key] = ("pending", nc)
    runner = _state[key]

    x = np.asarray(x, np.float32)
    xpad = np.zeros((NPAD, ROW), np.float32)
    xpad[:N, 0:D] = x
    xpad[:, D] = 1.0
    xfull = xpad.astype(BF16)

    wa1, wv1, ws1 = _prep_weights(*weights[0])
    wa2, wv2, ws2 = _prep_weights(*weights[1])
    iota = np.broadcast_to(np.arange(128, dtype=np.float32), (128, 128))
    iota = iota.astype(BF16)
    hot4 = np.zeros((128, H * H), np.float32)
    hot4[D, :] = np.eye(H, dtype=np.float32).ravel()
    hot4 = hot4.astype(BF16)

    in_maps = []
    for c in range(NC):
        in_maps.append({
            "xfull": xfull,
            "xsh": xfull[c * SH:(c + 1) * SH],
            "isrc": isrc[c], "slot": slot[c],
            "iota64": iota, "hot4": hot4,
            "wa1": wa1, "wv1": wv1, "ws1": ws1,
            "wa2": wa2, "wv2": wv2, "ws2": ws2,
        })
    if isinstance(runner, tuple):
        _warmup(t_h, in_maps)
        runner = _Runner(runner[1])
        _state[key] = runner
    return runner, in_maps


def _kernel_bass(x, edge_index, weights):
    runner, in_maps = _prepare(x, edge_index, weights)
    dev_in = runner.device_inputs(in_maps)
    outs = runner.run(dev_in)
    full = outs[0].reshape(NC * SH, D)
    return full[:N].copy()


# ---------------------------- JAX fallback ----------------------------------
def _kernel_jax(x, edge_index, weights):
    import jax
    import jax.numpy as jnp
    from jax.sharding import Mesh, NamedSharding, PartitionSpec as P
    try:
        from jax.experimental.shard_map import shard_map
    except ImportError:
        from jax import shard_map

    E = edge_index.shape[1]
    M = NC
    mesh = Mesh(np.array(jax.devices()[:M]), ('x',))
    rep = NamedSharding(mesh, P())
    esh = NamedSharding(mesh, P('x'))
    inv = np.float32(1.0 / np.sqrt(D))

    def smap(fn, in_specs, out_specs):
        return jax.jit(shard_map(fn, mesh=mesh, in_specs=in_specs,
                                 out_specs=out_specs))

    J = {}
    J['dense'] = smap(lambda x_, Wqkv, bqkv, Ws, bs:
                      tuple(jnp.split(x_ @ Wqkv + bqkv, 3, axis=1))
                      + (x_ @ Ws + bs,), (P(),) * 5, (P(), P(), P(), P()))
    J['gather'] = smap(lambda t, i: jnp.take(t, i, axis=0),
                       (P(), P('x')), P('x'))
    J['dot'] = smap(lambda a, b: (a * b).reshape(-1, H, D).sum(-1) * inv,
                    (P('x'), P('x')), P('x'))
    J['exp'] = smap(lambda a: jnp.exp(a), (P('x'),), P('x'))
    J['segsum'] = smap(lambda v, i: jax.lax.psum(
        jax.ops.segment_sum(v, i, num_segments=N), 'x'),
        (P('x'), P('x')), P())
    J['norm'] = smap(lambda ex, den, i: ex / (den[i] + 1e-16),
                     (P('x'), P(), P('x')), P('x'))
    J['msg'] = smap(lambda vs, at: vs * jnp.repeat(at, D, axis=1),
                    (P('x'), P('x')), P('x'))
    J['out'] = smap(lambda agg, skip: agg.reshape(N, H, D).mean(axis=1) + skip,
                    (P(), P()), P())
    J['relu'] = smap(lambda h: jax.nn.relu(h), (P(),), P())

    def lyr(x_d, s, d, Wqkv, bqkv, Ws, bs):
        q, k, v, skip = J['dense'](x_d, Wqkv, bqkv, Ws, bs)
        alpha = J['dot'](J['gather'](q, d), J['gather'](k, s))
        ex = J['exp'](alpha)
        den = J['segsum'](ex, d)
        attn = J['norm'](ex, den, d)
        msg = J['msg'](J['gather'](v, s), attn)
        return J['out'](J['segsum'](msg, d), skip)

    ei = np.asarray(edge_index)
    s = jax.device_put(jnp.asarray(ei[0]), esh)
    d = jax.device_put(jnp.asarray(ei[1]), esh)

    def prep(Wq, bq, Wk, bk, Wv, bv, Ws, bs):
        Wqkv = np.concatenate([Wq, Wk, Wv], axis=1)
        bqkv = np.concatenate([bq, bk, bv])
        return (jax.device_put(jnp.asarray(Wqkv), rep),
                jax.device_put(jnp.asarray(bqkv), rep),
                jax.device_put(jnp.asarray(Ws), rep),
                jax.device_put(jnp.asarray(bs), rep))

    W1 = prep(*weights[0])
    W2 = prep(*weights[1])
    x_d = jax.device_put(jnp.asarray(np.asarray(x)), rep)
    h = lyr(x_d, s, d, *W1)
    h = J['relu'](h)
    out = lyr(h, s, d, *W2)
    return np.asarray(jax.device_get(out)).astype(np.float32)


def kernel(x, edge_index, Wq1, bq1, Wk1, bk1, Wv1, bv1, Ws1, bs1,
           Wq2, bq2, Wk2, bk2, Wv2, bv2, Ws2, bs2):
    weights = ((Wq1, bq1, Wk1, bk1, Wv1, bv1, Ws1, bs1),
               (Wq2, bq2, Wk2, bk2, Wv2, bv2, Ws2, bs2))
    edge_index = np.asarray(edge_index)
    try:
        return _kernel_bass(np.asarray(x), edge_index, weights)
    except Exception as e:  # pragma: no cover - safety net
        import traceback
        traceback.print_exc()
        print(f"[kernel] bass path failed ({e!r}); falling back to JAX")
        return _kernel_jax(np.asarray(x), edge_index, weights)
